# revision 26
# baseline (speedup 1.0000x reference)
"""Causal multi-head self-attention on 8 Trainium2 NeuronCores.

Sharding: 4 batches x 2 head-groups (8 heads each). Core c = (b, g) with
b = c // 2, g = c % 2. Each core computes QKV projections for its weight
row-slice, attention for its 8 heads, and a partial out-projection
(Megatron row-parallel). Host sums the two partials per batch and adds
bo + bv @ Wo.T (the V-bias folds out of attention exactly: softmax rows
sum to 1).

The instruction stream software-pipelines phases: QKV-projection matmul
groups for later s-blocks and out-projection groups for earlier s-blocks
are emitted between attention chunks, placed so that tensor-engine work
fills the phases where attention is paced by the scalar-engine exp.

Softmax normalization uses a Z-broadcast trick: the attn@V stationary
operand is [ones (64 cols) | v_head (64 cols)], so the same matmul pass
that produces the attention output on PSUM partitions 64-127 also lands
the softmax denominator Z broadcast across partitions 0-63 for zero
extra PE cycles. Normalization is then a DVE reciprocal (all base-0;
the custom DVE ops cannot shift partition bases) plus one plain multiply
per head (plain DVE ops may read any partition window and write either
quadrant-pair, so the h1 multiply writes partitions 64-127 directly).

All shapes hardcoded for x [4, 2048, 1024], 16 heads, head_dim 64, fp32.
"""

import sys
import numpy as np

if "/opt/trn_rl_repo" not in sys.path:
    sys.path.insert(0, "/opt/trn_rl_repo")

B = 4
S = 2048
D = 1024
HG = 2            # head groups (cores per batch)
NHL = 8           # heads per core
DH = 64
DG = NHL * DH     # 512 feature dims per core
SB = 512          # s-block
NSB = S // SB     # 4
SCALE = 0.125     # 1/sqrt(64)

_CACHE = {}


def _build_nc():
    import concourse.bass as bass
    import concourse.bacc as bacc
    import concourse.tile as tile
    from concourse import mybir
    from contextlib import ExitStack

    f32 = mybir.dt.float32
    bf16 = mybir.dt.bfloat16
    AF = mybir.ActivationFunctionType
    ts = bass.ts

    nc = bacc.Bacc(None, target_bir_lowering=False)

    # ec-major packed layouts (per-partition-contiguous rows); loaded in
    # 2-ec chunks so projection matmuls pipeline with the transfers
    xt_d = nc.dram_tensor("xt", [128, 4, 8 * SB], bf16, kind="ExternalInput")
    wqt_d = nc.dram_tensor("wqt", [128, 8 * DG], bf16, kind="ExternalInput")
    wkt_d = nc.dram_tensor("wkt", [128, 8 * DG], bf16, kind="ExternalInput")
    wvt_d = nc.dram_tensor("wvt", [128, 8 * DG], bf16, kind="ExternalInput")
    wot_d = nc.dram_tensor("wot", [128, 4 * D], bf16, kind="ExternalInput")
    bqc_d = nc.dram_tensor("bqc", [128, 4], f32, kind="ExternalInput")
    bkc_d = nc.dram_tensor("bkc", [128, 4], f32, kind="ExternalInput")
    out_d = nc.dram_tensor("out", [S, D], bf16, kind="ExternalOutput")

    with tile.TileContext(nc) as tc, ExitStack() as ctx:
        consts = ctx.enter_context(tc.tile_pool(name="consts", bufs=1))
        cache = ctx.enter_context(tc.tile_pool(name="cache", bufs=1))
        xt_pool = ctx.enter_context(tc.tile_pool(name="xtp", bufs=2))
        qt_pool = ctx.enter_context(tc.tile_pool(name="qtp", bufs=3))
        work = ctx.enter_context(tc.tile_pool(name="work", bufs=1))
        ppool = ctx.enter_context(tc.tile_pool(name="pp", bufs=2, space="PSUM"))
        pscore = ctx.enter_context(tc.tile_pool(name="ps", bufs=2, space="PSUM"))
        pout2 = ctx.enter_context(tc.tile_pool(name="po", bufs=2, space="PSUM"))

        # ---- HAM warmup: matmuls on a junk tile keep the PE busy from
        # preamble-end (~7.3us) until the first projection matmuls flow,
        # so HAM un-throttles (K=8/8) before real work.
        junk_t = consts.tile([128, 512], bf16)
        nc.vector.memset(junk_t, 0.0)
        for w in range(48):
            # warmups from the score-PSUM ring bridge the whole input-DMA
            # window (~7.3-20us) so the first real matmuls run at K=8/8
            # instead of re-throttled 1.2GHz
            pwarm = pscore.tile([128, 2, SB], f32, tag="ps", name=f"pwarm_{w}")
            nc.tensor.matmul(
                pwarm[:, 0, :], junk_t[:, 0:128], junk_t[:, 0:512],
                start=True, stop=True,
            )

        # ---- weights / constants in SBUF ----
        wq_t = consts.tile([128, 8 * DG], bf16)
        wk_t = consts.tile([128, 8 * DG], bf16)
        wv_t = consts.tile([128, 8 * DG], bf16)
        wo_t = consts.tile([128, 4 * D], bf16)
        bqc_t = consts.tile([128, 4], f32)
        bkc_t = consts.tile([128, 4], f32)

        xt_tiles = {}

        def emit_xt_load(sb):
            # single DMA (128 x 8KB lines) on sync: keeps gpsimd free for the
            # causal-mask selects; the xt-pool slot WAR naturally delays this
            # transfer until the previous-but-one block's readers finish.
            xt_sb = xt_pool.tile([128, 8 * SB], bf16, tag="xt", name=f"xt_{sb}")
            nc.sync.dma_start(xt_sb[:, :], xt_d[:, sb, :])
            xt_tiles[sb] = xt_sb

        # startup-critical loads only (xt0 + QKV weights), 2-ec chunks
        # round-robined so transfers pipeline into the first matmul groups;
        # xt1/xt2/wo are issued inside the s-block loop.
        nc.sync.dma_start(bqc_t[:, :], bqc_d[:, :])
        nc.sync.dma_start(bkc_t[:, :], bkc_d[:, :])
        xt_0 = xt_pool.tile([128, 8 * SB], bf16, tag="xt", name="xt_0")
        xt_tiles[0] = xt_0
        # xt0 halves lead both queues (every projection streams xt), then
        # dc-major wq/wk chunks: chunk c unlocks head-pair c's projections
        nc.sync.dma_start(xt_0[:, 0 : 4 * SB], xt_d[:, 0, 0 : 4 * SB])
        nc.scalar.dma_start(xt_0[:, 4 * SB : 8 * SB], xt_d[:, 0, 4 * SB : 8 * SB])
        for c in range(4):
            sl = slice(c * 1024, (c + 1) * 1024)
            nc.sync.dma_start(wq_t[:, sl], wqt_d[:, sl])
            nc.scalar.dma_start(wk_t[:, sl], wkt_d[:, sl])
            nc.gpsimd.dma_start(wv_t[:, c * 2 * DG : (c + 1) * 2 * DG],
                                wvt_d[:, c * 2 * DG : (c + 1) * 2 * DG])

        # ---- persistent K/V caches ----
        kt_all = cache.tile([128, 4, S], bf16)       # [d within pair chunk, pair, t]
        # [t within chunk, tchunk, head, ones(64)|v(64)]
        v_aug = cache.tile([128, 16, NHL, 128], bf16)
        for tcm in range(16):
            nc.vector.memset(v_aug[:, tcm, :, 0:DH], 1.0)

        qt_tiles = {}
        ao_tiles = {}

        # ---- work-item emitters (each is one PSUM-group of tensor work) ----
        def emit_proj_q(sb, dc):
            xt_sb = xt_tiles[sb]
            if dc == 0:
                qt_tiles[sb] = qt_pool.tile(
                    [128, 4, SB], bf16, tag="qt", name=f"qt_{sb}"
                )
            qt_sb = qt_tiles[sb]
            pq = ppool.tile([128, SB], f32, tag="pp", name=f"pq_{sb}_{dc}")
            for ec in range(8):
                nc.tensor.matmul(
                    pq, wq_t[:, dc * 1024 + ec * 128 : dc * 1024 + ec * 128 + 128],
                    xt_sb[:, ec * SB : (ec + 1) * SB],
                    start=(ec == 0), stop=(ec == 7),
                )
            # bias-add on scalar: keeps the vector queue free so fill-group
            # PSUM slots release promptly (ring-release stalls otherwise)
            nc.scalar.activation(
                qt_sb[:, dc, :], pq, AF.Identity, bias=bqc_t[:, dc : dc + 1]
            )

        def emit_proj_k(sb, dc):
            xt_sb = xt_tiles[sb]
            s0 = sb * SB
            pk = ppool.tile([128, SB], f32, tag="pp", name=f"pk_{sb}_{dc}")
            for ec in range(8):
                nc.tensor.matmul(
                    pk, wk_t[:, dc * 1024 + ec * 128 : dc * 1024 + ec * 128 + 128],
                    xt_sb[:, ec * SB : (ec + 1) * SB],
                    start=(ec == 0), stop=(ec == 7),
                )
            nc.scalar.activation(
                kt_all[:, dc, s0 : s0 + SB], pk, AF.Identity,
                bias=bkc_t[:, dc : dc + 1],
            )

        def emit_proj_v(sb, tsub):
            xt_sb = xt_tiles[sb]
            tcg = 4 * sb + tsub
            pv = ppool.tile([128, NHL, DH], f32, tag="pp", name=f"pv_{sb}_{tsub}")
            for ec in range(8):
                nc.tensor.matmul(
                    pv, xt_sb[:, ec * SB + tsub * 128 : ec * SB + tsub * 128 + 128],
                    wv_t[:, ec * DG : (ec + 1) * DG],
                    start=(ec == 0), stop=(ec == 7),
                )
            nc.vector.tensor_copy(v_aug[:, tcg, :, DH:128], pv[:, :, :])

        def emit_outproj(sb, sc, oh, idx=0):
            s0 = sb * SB
            ao = ao_tiles[sb]
            last = sb == NSB - 1
            # the final two groups go in 256-col halves so the copy+store of
            # half A overlaps the matmuls of half B (shorter serial tail)
            halves = 2 if (last and idx >= 6) else 1
            hw = 512 // halves
            for h2 in range(halves):
                o0 = oh * 512 + h2 * hw
                pool = pout2 if (last and (idx + h2) % 2 == 1) else ppool
                po = pool.tile(
                    [128, hw], f32, tag=("po" if pool is pout2 else "pp"),
                    name=f"pop_{sb}_{sc}_{oh}_{h2}",
                )
                for p in range(4):
                    nc.tensor.matmul(
                        po,
                        ao[p][:, ts(sc, 128)],
                        wo_t[:, p * D + o0 : p * D + o0 + hw],
                        start=(p == 0), stop=(p == 3),
                    )
                po_sb = work.tile([128, hw], bf16, tag="posb", bufs=3)
                if last:
                    # tail: alternate scalar/vector copies so they pipeline
                    if (idx + h2) % 2 == 0:
                        nc.scalar.copy(po_sb, po)
                    else:
                        nc.vector.tensor_copy(po_sb, po)
                else:
                    nc.vector.tensor_copy(po_sb, po)
                (nc.gpsimd if (last and idx % 2 == 1) else nc.sync).dma_start(
                    out_d[s0 + 128 * sc : s0 + 128 * (sc + 1), o0 : o0 + hw],
                    po_sb,
                )

        def proj_items(sb):
            items = []
            for dc in range(4):
                items.append(lambda sb=sb, dc=dc: emit_proj_q(sb, dc))
            for dc in range(4):
                items.append(lambda sb=sb, dc=dc: emit_proj_k(sb, dc))
            for tsub in range(4):
                items.append(lambda sb=sb, tsub=tsub: emit_proj_v(sb, tsub))
            return items

        def outproj_items(sb):
            return [
                lambda sb=sb, sc=sc, oh=oh, i=i: emit_outproj(sb, sc, oh, i)
                for i, (sc, oh) in enumerate(
                    (sc, oh) for sc in range(4) for oh in range(2)
                )
            ]

        # full proj(0) ahead of attention(0): during the input-DMA window
        # the PE has ~20us of projection work that pipelines with the 2-ec
        # chunk arrivals, so it never idles waiting for attention deps.
        p0 = proj_items(0)
        for it in [p0[0], p0[4], p0[8], p0[9], p0[10], p0[11],
                   p0[1], p0[5], p0[2], p0[6], p0[3], p0[7]]:
            it()

        # fill-schedule per attention phase: projections go to the early
        # (exp-light, PE-idle) phases, all out-projections to the exp-bound
        # last phase.
        p3 = proj_items(3)
        fills = {
            0: proj_items(1),
            1: proj_items(2) + p3[:2],
            2: p3[2:],
            3: [],
        }

        for sb in range(NSB):
            s0 = sb * SB
            nkc = 4 * sb + 4
            qt_sb = qt_tiles[sb]

            if sb == 0:
                emit_xt_load(1)
                emit_xt_load(2)
                nc.gpsimd.dma_start(wo_t[:, :], wot_d[:, :])
            elif sb == 1:
                emit_xt_load(3)
            fill = fills[sb]
            if sb == 3:
                fill = (
                    fill + outproj_items(0) + outproj_items(1)
                    + outproj_items(2)[:3]
                )
            total_chunks = 4 * nkc
            # even spread: emit fill item i after chunk floor((i+1)*T/(n+1));
            # sb=0 fills need xt1, which lands after the critical loads - bias
            # them into the second half so they never block attention(0)
            if sb == 0 and fill:
                emit_at = [
                    8 + (i + 1) * (total_chunks - 8) // (len(fill) + 1)
                    for i in range(len(fill))
                ]
            else:
                emit_at = [
                    (i + 1) * total_chunks // (len(fill) + 1)
                    for i in range(len(fill))
                ]
            fill_i = 0
            chunk_i = 0

            ao_tiles[sb] = []
            for p in range(4):
                out2 = [
                    pout2.tile([128, SB], f32, tag="po", name=f"out2_{hh}")
                    for hh in range(2)
                ]
                prev = None  # (exp tile, col offset, key chunk)
                for kc in range(nkc):
                    j = kc - 4 * sb  # >= 0 on diagonal chunks
                    c0 = 128 * j if j > 0 else 0
                    ps_t = pscore.tile([128, 2, SB], f32, tag="ps")
                    for hh in range(2):
                        r0 = 64 * hh
                        nc.tensor.matmul(
                            ps_t[:, hh, c0:SB],
                            kt_all[r0 : r0 + 64, p, ts(kc, 128)],
                            qt_sb[r0 : r0 + 64, p, c0:SB],
                            start=True, stop=True,
                        )
                    if prev is not None:
                        pex, pc0, pkc = prev
                        for hh in range(2):
                            nc.tensor.matmul(
                                out2[hh][:, pc0:SB],
                                v_aug[:, pkc, 2 * p + hh, :],
                                pex[:, hh, pc0:SB],
                                start=(pkc == 0), stop=False,
                            )
                    ex = work.tile([128, 2, SB], bf16, tag="expt", bufs=4)
                    nc.scalar.activation(
                        ex[:, :, c0:SB], ps_t[:, :, c0:SB], AF.Exp, scale=SCALE
                    )
                    if j >= 0:
                        # causal mask: zero ex[k, hh, q] where q < k within the
                        # 128x128 diagonal block (iota = col - chan, keep >= 0)
                        nc.gpsimd.affine_select(
                            out=ex[:, :, c0 : c0 + 128],
                            in_=ex[:, :, c0 : c0 + 128],
                            compare_op=mybir.AluOpType.is_ge,
                            fill=0.0,
                            base=0,
                            pattern=[[0, 2], [1, 128]],
                            channel_multiplier=-1,
                        )
                    prev = (ex, c0, kc)
                    chunk_i += 1
                    while fill_i < len(fill) and chunk_i >= emit_at[fill_i]:
                        fill[fill_i]()
                        fill_i += 1
                # final attn@V for the last key chunk
                pex, pc0, pkc = prev
                for hh in range(2):
                    nc.tensor.matmul(
                        out2[hh][:, pc0:SB],
                        v_aug[:, pkc, 2 * p + hh, :],
                        pex[:, hh, pc0:SB],
                        start=(pkc == 0), stop=True,
                    )

                # ---- normalization: Z sits broadcast on partitions 0-63 of
                # the same PSUM tile (ones block leads the AV stationary).
                ao_p = work.tile([128, SB], bf16, tag=f"ao{p}", bufs=4)
                for hh in range(2):
                    rbc = work.tile([64, SB], f32, tag="rbc", bufs=2)
                    nc.vector.reciprocal_approx_fast(rbc, out2[hh][0:64, :])
                    nc.vector.tensor_mul(
                        ao_p[64 * hh : 64 * hh + 64, :], out2[hh][64:128, :], rbc
                    )
                ao_tiles[sb].append(ao_p)
                for _ in range(2):
                    if fill_i < len(fill):
                        fill[fill_i]()
                        fill_i += 1

            while fill_i < len(fill):
                fill[fill_i]()
                fill_i += 1

        # tail: four held-back outproj(2) groups (no dependency on the last
        # norm) keep the PE busy while the DVE normalizes p=3, then the final
        # out-projections follow.
        for it in outproj_items(2)[3:] + outproj_items(NSB - 1):
            it()

    nc.compile()
    return nc


def _pack8(a, dtype):
    # [1024, N] row-major -> [128, 8*N]: row p holds chunks
    # {p, 128+p, ..., 896+p} concatenated (contiguous DMA lines per row)
    n = a.shape[1]
    return np.ascontiguousarray(
        a.reshape(8, 128, n).transpose(1, 0, 2).reshape(128, 8 * n)
    ).astype(dtype)


def _pack8_dc(a, dtype):
    # [1024, 512] -> [128, 4096] dc-major: [p, dc*1024 + ec*128 + r]
    return np.ascontiguousarray(
        a.reshape(8, 128, 4, 128).transpose(1, 2, 0, 3).reshape(128, 4096)
    ).astype(dtype)


def _pack_xt(xt, dtype):
    # x[b].T [1024, 2048] -> [128, 4, 4096]: [p, sb, ec*512+i]
    return np.ascontiguousarray(
        xt.reshape(8, 128, 4, 512).transpose(1, 2, 0, 3).reshape(128, 4, 4096)
    ).astype(dtype)


def _prepare_core_inputs(x, Wq, bq, Wk, bk, Wv):
    """Build per-core input maps. Core c: b = c // 2, g = c % 2."""
    import ml_dtypes

    BF = ml_dtypes.bfloat16
    maps = []
    xt = [_pack_xt(np.ascontiguousarray(x[b].T), BF) for b in range(B)]
    wq_s, wk_s, wv_s, bq_s, bk_s = [], [], [], [], []
    for g in range(HG):
        sl = slice(g * DG, (g + 1) * DG)
        wq_s.append(_pack8_dc(np.ascontiguousarray(Wq[sl, :].T), BF))
        wk_s.append(_pack8_dc(np.ascontiguousarray(Wk[sl, :].T), BF))
        wv_s.append(_pack8(np.ascontiguousarray(Wv[sl, :].T), BF))
        # per-dim bias columns: [128, 4] = bias[dc*128 + r] at [r, dc]
        bq_s.append(np.ascontiguousarray(bq[sl].reshape(4, 128).T).astype(np.float32))
        bk_s.append(np.ascontiguousarray(bk[sl].reshape(4, 128).T).astype(np.float32))
    for c in range(B * HG):
        b, g = c // HG, c % HG
        maps.append({
            "xt": xt[b],
            "wqt": wq_s[g], "wkt": wk_s[g], "wvt": wv_s[g],
            "wot": None,  # filled by caller (needs Wo)
            "bqc": bq_s[g], "bkc": bk_s[g],
        })
    return maps


def kernel(x, Wq, bq, Wk, bk, Wv, bv, Wo, bo):
    from concourse.bass_utils import run_bass_kernel_spmd

    x = np.asarray(x, dtype=np.float32)
    Wq, bq = np.asarray(Wq, np.float32), np.asarray(bq, np.float32)
    Wk, bk = np.asarray(Wk, np.float32), np.asarray(bk, np.float32)
    Wv, bv = np.asarray(Wv, np.float32), np.asarray(bv, np.float32)
    Wo, bo = np.asarray(Wo, np.float32), np.asarray(bo, np.float32)

    if "nc" not in _CACHE:
        _CACHE["nc"] = _build_nc()
    nc = _CACHE["nc"]

    maps = _prepare_core_inputs(x, Wq, bq, Wk, bk, Wv)
    wot = [_pack_wo(Wo, g) for g in range(HG)]
    for c in range(B * HG):
        maps[c]["wot"] = wot[c % HG]

    res = run_bass_kernel_spmd(nc, maps, list(range(B * HG)))
    _CACHE["last_results"] = res

    # V-bias folds out of attention exactly (softmax rows sum to 1)
    bo_eff = bo + bv @ Wo.T

    out = np.empty((B, S, D), dtype=np.float32)
    for b in range(B):
        out[b] = (
            res.results[2 * b]["out"].astype(np.float32)
            + res.results[2 * b + 1]["out"].astype(np.float32)
            + bo_eff
        )
    return out


def _pack_wo(Wo, g):
    import ml_dtypes

    w = np.ascontiguousarray(Wo.T[g * DG : (g + 1) * DG, :])  # [512, 1024]
    return np.ascontiguousarray(
        w.reshape(4, 128, D).transpose(1, 0, 2).reshape(128, 4 * D)
    ).astype(ml_dtypes.bfloat16)


# revision 27
# speedup vs baseline: 1.0830x; 1.0830x over previous
"""Causal multi-head self-attention on 8 Trainium2 NeuronCores.

Sharding: 4 batches x 2 head-groups (8 heads each). Core c = (b, g) with
b = c // 2, g = c % 2. Each core computes QKV projections for its weight
row-slice, attention for its 8 heads, and a partial out-projection
(Megatron row-parallel). Host sums the two partials per batch and adds
bo + bv @ Wo.T (the V-bias folds out of attention exactly: softmax rows
sum to 1).

The instruction stream software-pipelines phases: QKV-projection matmul
groups for later s-blocks and out-projection groups for earlier s-blocks
are emitted between attention chunks, placed so that tensor-engine work
fills the phases where attention is paced by the scalar-engine exp.

Softmax normalization uses a Z-broadcast trick: the attn@V stationary
operand is [ones (64 cols) | v_head (64 cols)], so the same matmul pass
that produces the attention output on PSUM partitions 64-127 also lands
the softmax denominator Z broadcast across partitions 0-63 for zero
extra PE cycles. Normalization is then a DVE reciprocal (all base-0;
the custom DVE ops cannot shift partition bases) plus one plain multiply
per head (plain DVE ops may read any partition window and write either
quadrant-pair, so the h1 multiply writes partitions 64-127 directly).

All shapes hardcoded for x [4, 2048, 1024], 16 heads, head_dim 64, fp32.
"""

import sys
import numpy as np

if "/opt/trn_rl_repo" not in sys.path:
    sys.path.insert(0, "/opt/trn_rl_repo")

B = 4
S = 2048
D = 1024
HG = 2            # head groups (cores per batch)
NHL = 8           # heads per core
DH = 64
DG = NHL * DH     # 512 feature dims per core
SB = 512          # s-block
NSB = S // SB     # 4
SCALE = 0.125     # 1/sqrt(64)

_CACHE = {}


def _build_nc():
    import concourse.bass as bass
    import concourse.bacc as bacc
    import concourse.tile as tile
    from concourse import mybir
    from contextlib import ExitStack

    f32 = mybir.dt.float32
    bf16 = mybir.dt.bfloat16
    AF = mybir.ActivationFunctionType
    ts = bass.ts

    nc = bacc.Bacc(None, target_bir_lowering=False)

    # ec-major packed layouts (per-partition-contiguous rows); loaded in
    # 2-ec chunks so projection matmuls pipeline with the transfers
    xt_d = nc.dram_tensor("xt", [128, 4, 8 * SB], bf16, kind="ExternalInput")
    wqt_d = nc.dram_tensor("wqt", [128, 8 * DG], bf16, kind="ExternalInput")
    wkt_d = nc.dram_tensor("wkt", [128, 8 * DG], bf16, kind="ExternalInput")
    wvt_d = nc.dram_tensor("wvt", [128, 8 * DG], bf16, kind="ExternalInput")
    wot_d = nc.dram_tensor("wot", [128, 4 * D], bf16, kind="ExternalInput")
    bqc_d = nc.dram_tensor("bqc", [128, 4], f32, kind="ExternalInput")
    bkc_d = nc.dram_tensor("bkc", [128, 4], f32, kind="ExternalInput")
    out_d = nc.dram_tensor("out", [S, D], bf16, kind="ExternalOutput")

    with tile.TileContext(nc) as tc, ExitStack() as ctx:
        consts = ctx.enter_context(tc.tile_pool(name="consts", bufs=1))
        cache = ctx.enter_context(tc.tile_pool(name="cache", bufs=1))
        xt_pool = ctx.enter_context(tc.tile_pool(name="xtp", bufs=2))
        qt_pool = ctx.enter_context(tc.tile_pool(name="qtp", bufs=3))
        work = ctx.enter_context(tc.tile_pool(name="work", bufs=1))
        ppool = ctx.enter_context(tc.tile_pool(name="pp", bufs=2, space="PSUM"))
        pscore = ctx.enter_context(tc.tile_pool(name="ps", bufs=2, space="PSUM"))
        pout2 = ctx.enter_context(tc.tile_pool(name="po", bufs=2, space="PSUM"))

        # ---- HAM warmup: matmuls on a junk tile keep the PE busy from
        # preamble-end (~7.3us) until the first projection matmuls flow,
        # so HAM un-throttles (K=8/8) before real work.
        junk_t = consts.tile([128, 512], bf16)
        nc.vector.memset(junk_t, 0.0)
        for w in range(48):
            # warmups from the score-PSUM ring bridge the whole input-DMA
            # window (~7.3-20us) so the first real matmuls run at K=8/8
            # instead of re-throttled 1.2GHz
            pwarm = pscore.tile([128, 2, SB], f32, tag="ps", name=f"pwarm_{w}")
            nc.tensor.matmul(
                pwarm[:, 0, :], junk_t[:, 0:128], junk_t[:, 0:512],
                start=True, stop=True,
            )

        # ---- weights / constants in SBUF ----
        wq_t = consts.tile([128, 8 * DG], bf16)
        wk_t = consts.tile([128, 8 * DG], bf16)
        wv_t = consts.tile([128, 8 * DG], bf16)
        wo_t = consts.tile([128, 4 * D], bf16)
        bqc_t = consts.tile([128, 4], f32)
        bkc_t = consts.tile([128, 4], f32)

        xt_tiles = {}

        def emit_xt_load(sb):
            # single DMA (128 x 8KB lines) on sync: keeps gpsimd free for the
            # causal-mask selects; the xt-pool slot WAR naturally delays this
            # transfer until the previous-but-one block's readers finish.
            xt_sb = xt_pool.tile([128, 8 * SB], bf16, tag="xt", name=f"xt_{sb}")
            nc.sync.dma_start(xt_sb[:, :], xt_d[:, sb, :])
            xt_tiles[sb] = xt_sb

        # startup-critical loads only (xt0 + QKV weights), 2-ec chunks
        # round-robined so transfers pipeline into the first matmul groups;
        # xt1/xt2/wo are issued inside the s-block loop.
        nc.sync.dma_start(bqc_t[:, :], bqc_d[:, :])
        nc.sync.dma_start(bkc_t[:, :], bkc_d[:, :])
        xt_0 = xt_pool.tile([128, 8 * SB], bf16, tag="xt", name="xt_0")
        xt_tiles[0] = xt_0
        # xt0 halves lead both queues (every projection streams xt), then
        # dc-major wq/wk chunks: chunk c unlocks head-pair c's projections
        nc.sync.dma_start(xt_0[:, 0 : 4 * SB], xt_d[:, 0, 0 : 4 * SB])
        nc.scalar.dma_start(xt_0[:, 4 * SB : 8 * SB], xt_d[:, 0, 4 * SB : 8 * SB])
        for c in range(4):
            sl = slice(c * 1024, (c + 1) * 1024)
            nc.sync.dma_start(wq_t[:, sl], wqt_d[:, sl])
            nc.scalar.dma_start(wk_t[:, sl], wkt_d[:, sl])
            nc.gpsimd.dma_start(wv_t[:, c * 2 * DG : (c + 1) * 2 * DG],
                                wvt_d[:, c * 2 * DG : (c + 1) * 2 * DG])

        # ---- persistent K/V caches ----
        kt_all = cache.tile([128, 4, S], bf16)       # [d within pair chunk, pair, t]
        # [t within chunk, tchunk, head, ones(64)|v(64)]
        v_aug = cache.tile([128, 16, NHL, 128], bf16)
        for tcm in range(16):
            nc.vector.memset(v_aug[:, tcm, :, 0:DH], 1.0)

        qt_tiles = {}
        ao_tiles = {}

        # ---- work-item emitters (each is one PSUM-group of tensor work) ----
        def emit_proj_q(sb, dc):
            xt_sb = xt_tiles[sb]
            if dc == 0:
                qt_tiles[sb] = qt_pool.tile(
                    [128, 4, SB], bf16, tag="qt", name=f"qt_{sb}"
                )
            qt_sb = qt_tiles[sb]
            pq = ppool.tile([128, SB], f32, tag="pp", name=f"pq_{sb}_{dc}")
            for ec in range(8):
                nc.tensor.matmul(
                    pq, wq_t[:, dc * 1024 + ec * 128 : dc * 1024 + ec * 128 + 128],
                    xt_sb[:, ec * SB : (ec + 1) * SB],
                    start=(ec == 0), stop=(ec == 7),
                )
            # bias-add on scalar: keeps the vector queue free so fill-group
            # PSUM slots release promptly (ring-release stalls otherwise)
            nc.scalar.activation(
                qt_sb[:, dc, :], pq, AF.Identity, bias=bqc_t[:, dc : dc + 1]
            )

        def emit_proj_k(sb, dc):
            xt_sb = xt_tiles[sb]
            s0 = sb * SB
            pk = ppool.tile([128, SB], f32, tag="pp", name=f"pk_{sb}_{dc}")
            for ec in range(8):
                nc.tensor.matmul(
                    pk, wk_t[:, dc * 1024 + ec * 128 : dc * 1024 + ec * 128 + 128],
                    xt_sb[:, ec * SB : (ec + 1) * SB],
                    start=(ec == 0), stop=(ec == 7),
                )
            nc.scalar.activation(
                kt_all[:, dc, s0 : s0 + SB], pk, AF.Identity,
                bias=bkc_t[:, dc : dc + 1],
            )

        def emit_proj_v(sb, tsub):
            xt_sb = xt_tiles[sb]
            tcg = 4 * sb + tsub
            pv = ppool.tile([128, NHL, DH], f32, tag="pp", name=f"pv_{sb}_{tsub}")
            for ec in range(8):
                nc.tensor.matmul(
                    pv, xt_sb[:, ec * SB + tsub * 128 : ec * SB + tsub * 128 + 128],
                    wv_t[:, ec * DG : (ec + 1) * DG],
                    start=(ec == 0), stop=(ec == 7),
                )
            nc.vector.tensor_copy(v_aug[:, tcg, :, DH:128], pv[:, :, :])

        def emit_outproj(sb, sc, oh, idx=0):
            s0 = sb * SB
            ao = ao_tiles[sb]
            last = sb == NSB - 1
            # the final two groups go in 256-col halves so the copy+store of
            # half A overlaps the matmuls of half B (shorter serial tail)
            halves = 2 if (last and idx >= 6) else 1
            hw = 512 // halves
            for h2 in range(halves):
                o0 = oh * 512 + h2 * hw
                pool = pout2 if (last and (idx + h2) % 2 == 1) else ppool
                po = pool.tile(
                    [128, hw], f32, tag=("po" if pool is pout2 else "pp"),
                    name=f"pop_{sb}_{sc}_{oh}_{h2}",
                )
                for p in range(4):
                    nc.tensor.matmul(
                        po,
                        ao[p][:, ts(sc, 128)],
                        wo_t[:, p * D + o0 : p * D + o0 + hw],
                        start=(p == 0), stop=(p == 3),
                    )
                po_sb = work.tile([128, hw], bf16, tag="posb", bufs=3)
                if last:
                    # tail: alternate scalar/vector copies so they pipeline
                    if (idx + h2) % 2 == 0:
                        nc.scalar.copy(po_sb, po)
                    else:
                        nc.vector.tensor_copy(po_sb, po)
                else:
                    nc.vector.tensor_copy(po_sb, po)
                (nc.gpsimd if (last and idx % 2 == 1) else nc.sync).dma_start(
                    out_d[s0 + 128 * sc : s0 + 128 * (sc + 1), o0 : o0 + hw],
                    po_sb,
                )

        def proj_items(sb):
            items = []
            for dc in range(4):
                items.append(lambda sb=sb, dc=dc: emit_proj_q(sb, dc))
            for dc in range(4):
                items.append(lambda sb=sb, dc=dc: emit_proj_k(sb, dc))
            for tsub in range(4):
                items.append(lambda sb=sb, tsub=tsub: emit_proj_v(sb, tsub))
            return items

        def outproj_items(sb):
            return [
                lambda sb=sb, sc=sc, oh=oh, i=i: emit_outproj(sb, sc, oh, i)
                for i, (sc, oh) in enumerate(
                    (sc, oh) for sc in range(4) for oh in range(2)
                )
            ]

        # full proj(0) ahead of attention(0): during the input-DMA window
        # the PE has ~20us of projection work that pipelines with the 2-ec
        # chunk arrivals, so it never idles waiting for attention deps.
        p0 = proj_items(0)
        for it in [p0[0], p0[4], p0[8], p0[9], p0[10], p0[11],
                   p0[1], p0[5], p0[2], p0[6], p0[3], p0[7]]:
            it()

        # fill-schedule per attention phase: projections go to the early
        # (exp-light, PE-idle) phases, all out-projections to the exp-bound
        # last phase.
        p3 = proj_items(3)
        fills = {
            0: proj_items(1),
            1: proj_items(2) + p3[:2],
            2: p3[2:],
            3: [],
        }

        for sb in range(NSB):
            s0 = sb * SB
            nkc = 4 * sb + 4
            qt_sb = qt_tiles[sb]

            if sb == 0:
                emit_xt_load(1)
                emit_xt_load(2)
                nc.gpsimd.dma_start(wo_t[:, :], wot_d[:, :])
            elif sb == 1:
                emit_xt_load(3)
            fill = fills[sb]
            if sb == 3:
                fill = (
                    fill + outproj_items(0) + outproj_items(1)
                    + outproj_items(2)[:4]
                )
            total_chunks = 4 * nkc
            # even spread: emit fill item i after chunk floor((i+1)*T/(n+1));
            # sb=0 fills need xt1, which lands after the critical loads - bias
            # them into the second half so they never block attention(0)
            if sb == 0 and fill:
                emit_at = [
                    8 + (i + 1) * (total_chunks - 8) // (len(fill) + 1)
                    for i in range(len(fill))
                ]
            else:
                emit_at = [
                    (i + 1) * total_chunks // (len(fill) + 1)
                    for i in range(len(fill))
                ]
            fill_i = 0
            chunk_i = 0

            ao_tiles[sb] = []
            for p in range(4):
                out2 = [
                    pout2.tile([128, SB], f32, tag="po", name=f"out2_{hh}")
                    for hh in range(2)
                ]
                prev = None  # (exp tile, col offset, key chunk)
                for kc in range(nkc):
                    j = kc - 4 * sb  # >= 0 on diagonal chunks
                    c0 = 128 * j if j > 0 else 0
                    ps_t = pscore.tile([128, 2, SB], f32, tag="ps")
                    for hh in range(2):
                        r0 = 64 * hh
                        nc.tensor.matmul(
                            ps_t[:, hh, c0:SB],
                            kt_all[r0 : r0 + 64, p, ts(kc, 128)],
                            qt_sb[r0 : r0 + 64, p, c0:SB],
                            start=True, stop=True,
                        )
                    if prev is not None:
                        pex, pc0, pkc = prev
                        for hh in range(2):
                            nc.tensor.matmul(
                                out2[hh][:, pc0:SB],
                                v_aug[:, pkc, 2 * p + hh, :],
                                pex[:, hh, pc0:SB],
                                start=(pkc == 0), stop=False,
                            )
                    ex = work.tile([128, 2, SB], bf16, tag="expt", bufs=4)
                    nc.scalar.activation(
                        ex[:, :, c0:SB], ps_t[:, :, c0:SB], AF.Exp, scale=SCALE
                    )
                    if j >= 0:
                        # causal mask: zero ex[k, hh, q] where q < k within the
                        # 128x128 diagonal block (iota = col - chan, keep >= 0)
                        nc.gpsimd.affine_select(
                            out=ex[:, :, c0 : c0 + 128],
                            in_=ex[:, :, c0 : c0 + 128],
                            compare_op=mybir.AluOpType.is_ge,
                            fill=0.0,
                            base=0,
                            pattern=[[0, 2], [1, 128]],
                            channel_multiplier=-1,
                        )
                    prev = (ex, c0, kc)
                    chunk_i += 1
                    while fill_i < len(fill) and chunk_i >= emit_at[fill_i]:
                        fill[fill_i]()
                        fill_i += 1
                # final attn@V for the last key chunk
                pex, pc0, pkc = prev
                for hh in range(2):
                    nc.tensor.matmul(
                        out2[hh][:, pc0:SB],
                        v_aug[:, pkc, 2 * p + hh, :],
                        pex[:, hh, pc0:SB],
                        start=(pkc == 0), stop=True,
                    )

                # ---- normalization: Z sits broadcast on partitions 0-63 of
                # the same PSUM tile (ones block leads the AV stationary).
                ao_p = work.tile([128, SB], bf16, tag=f"ao{p}", bufs=4)
                for hh in range(2):
                    rbc = work.tile([64, SB], f32, tag="rbc", bufs=2)
                    nc.vector.reciprocal_approx_fast(rbc, out2[hh][0:64, :])
                    nc.vector.tensor_mul(
                        ao_p[64 * hh : 64 * hh + 64, :], out2[hh][64:128, :], rbc
                    )
                ao_tiles[sb].append(ao_p)
                for _ in range(2):
                    if fill_i < len(fill):
                        fill[fill_i]()
                        fill_i += 1

            while fill_i < len(fill):
                fill[fill_i]()
                fill_i += 1

        # tail: four held-back outproj(2) groups (no dependency on the last
        # norm) keep the PE busy while the DVE normalizes p=3, then the final
        # out-projections follow.
        for it in outproj_items(2)[4:] + outproj_items(NSB - 1):
            it()

    nc.compile()
    return nc


def _pack8(a, dtype):
    # [1024, N] row-major -> [128, 8*N]: row p holds chunks
    # {p, 128+p, ..., 896+p} concatenated (contiguous DMA lines per row)
    n = a.shape[1]
    return np.ascontiguousarray(
        a.reshape(8, 128, n).transpose(1, 0, 2).reshape(128, 8 * n)
    ).astype(dtype)


def _pack8_dc(a, dtype):
    # [1024, 512] -> [128, 4096] dc-major: [p, dc*1024 + ec*128 + r]
    return np.ascontiguousarray(
        a.reshape(8, 128, 4, 128).transpose(1, 2, 0, 3).reshape(128, 4096)
    ).astype(dtype)


def _pack_xt(xt, dtype):
    # x[b].T [1024, 2048] -> [128, 4, 4096]: [p, sb, ec*512+i]
    return np.ascontiguousarray(
        xt.reshape(8, 128, 4, 512).transpose(1, 2, 0, 3).reshape(128, 4, 4096)
    ).astype(dtype)


def _prepare_core_inputs(x, Wq, bq, Wk, bk, Wv):
    """Build per-core input maps. Core c: b = c // 2, g = c % 2."""
    import ml_dtypes

    BF = ml_dtypes.bfloat16
    maps = []
    xt = [_pack_xt(np.ascontiguousarray(x[b].T), BF) for b in range(B)]
    wq_s, wk_s, wv_s, bq_s, bk_s = [], [], [], [], []
    for g in range(HG):
        sl = slice(g * DG, (g + 1) * DG)
        wq_s.append(_pack8_dc(np.ascontiguousarray(Wq[sl, :].T), BF))
        wk_s.append(_pack8_dc(np.ascontiguousarray(Wk[sl, :].T), BF))
        wv_s.append(_pack8(np.ascontiguousarray(Wv[sl, :].T), BF))
        # per-dim bias columns: [128, 4] = bias[dc*128 + r] at [r, dc]
        bq_s.append(np.ascontiguousarray(bq[sl].reshape(4, 128).T).astype(np.float32))
        bk_s.append(np.ascontiguousarray(bk[sl].reshape(4, 128).T).astype(np.float32))
    for c in range(B * HG):
        b, g = c // HG, c % HG
        maps.append({
            "xt": xt[b],
            "wqt": wq_s[g], "wkt": wk_s[g], "wvt": wv_s[g],
            "wot": None,  # filled by caller (needs Wo)
            "bqc": bq_s[g], "bkc": bk_s[g],
        })
    return maps


def kernel(x, Wq, bq, Wk, bk, Wv, bv, Wo, bo):
    from concourse.bass_utils import run_bass_kernel_spmd

    x = np.asarray(x, dtype=np.float32)
    Wq, bq = np.asarray(Wq, np.float32), np.asarray(bq, np.float32)
    Wk, bk = np.asarray(Wk, np.float32), np.asarray(bk, np.float32)
    Wv, bv = np.asarray(Wv, np.float32), np.asarray(bv, np.float32)
    Wo, bo = np.asarray(Wo, np.float32), np.asarray(bo, np.float32)

    if "nc" not in _CACHE:
        _CACHE["nc"] = _build_nc()
    nc = _CACHE["nc"]

    maps = _prepare_core_inputs(x, Wq, bq, Wk, bk, Wv)
    wot = [_pack_wo(Wo, g) for g in range(HG)]
    for c in range(B * HG):
        maps[c]["wot"] = wot[c % HG]

    res = run_bass_kernel_spmd(nc, maps, list(range(B * HG)))
    _CACHE["last_results"] = res

    # V-bias folds out of attention exactly (softmax rows sum to 1)
    bo_eff = bo + bv @ Wo.T

    out = np.empty((B, S, D), dtype=np.float32)
    for b in range(B):
        out[b] = (
            res.results[2 * b]["out"].astype(np.float32)
            + res.results[2 * b + 1]["out"].astype(np.float32)
            + bo_eff
        )
    return out


def _pack_wo(Wo, g):
    import ml_dtypes

    w = np.ascontiguousarray(Wo.T[g * DG : (g + 1) * DG, :])  # [512, 1024]
    return np.ascontiguousarray(
        w.reshape(4, 128, D).transpose(1, 0, 2).reshape(128, 4 * D)
    ).astype(ml_dtypes.bfloat16)


# revision 28
# speedup vs baseline: 1.0877x; 1.0043x over previous
"""Causal multi-head self-attention on 8 Trainium2 NeuronCores.

Sharding: 4 batches x 2 head-groups (8 heads each). Core c = (b, g) with
b = c // 2, g = c % 2. Each core computes QKV projections for its weight
row-slice, attention for its 8 heads, and a partial out-projection
(Megatron row-parallel). Host sums the two partials per batch and adds
bo + bv @ Wo.T (the V-bias folds out of attention exactly: softmax rows
sum to 1).

The instruction stream software-pipelines phases: QKV-projection matmul
groups for later s-blocks and out-projection groups for earlier s-blocks
are emitted between attention chunks, placed so that tensor-engine work
fills the phases where attention is paced by the scalar-engine exp.

Softmax normalization uses a Z-broadcast trick: the attn@V stationary
operand is [ones (64 cols) | v_head (64 cols)], so the same matmul pass
that produces the attention output on PSUM partitions 64-127 also lands
the softmax denominator Z broadcast across partitions 0-63 for zero
extra PE cycles. Normalization is then a DVE reciprocal (all base-0;
the custom DVE ops cannot shift partition bases) plus one plain multiply
per head (plain DVE ops may read any partition window and write either
quadrant-pair, so the h1 multiply writes partitions 64-127 directly).

All shapes hardcoded for x [4, 2048, 1024], 16 heads, head_dim 64, fp32.
"""

import sys
import numpy as np

if "/opt/trn_rl_repo" not in sys.path:
    sys.path.insert(0, "/opt/trn_rl_repo")

B = 4
S = 2048
D = 1024
HG = 2            # head groups (cores per batch)
NHL = 8           # heads per core
DH = 64
DG = NHL * DH     # 512 feature dims per core
SB = 512          # s-block
NSB = S // SB     # 4
SCALE = 0.125     # 1/sqrt(64)

_CACHE = {}


def _build_nc():
    import concourse.bass as bass
    import concourse.bacc as bacc
    import concourse.tile as tile
    from concourse import mybir
    from contextlib import ExitStack

    f32 = mybir.dt.float32
    bf16 = mybir.dt.bfloat16
    AF = mybir.ActivationFunctionType
    ts = bass.ts

    nc = bacc.Bacc(None, target_bir_lowering=False)

    # ec-major packed layouts (per-partition-contiguous rows); loaded in
    # 2-ec chunks so projection matmuls pipeline with the transfers
    xt_d = nc.dram_tensor("xt", [128, 4, 8 * SB], bf16, kind="ExternalInput")
    wqt_d = nc.dram_tensor("wqt", [128, 8 * DG], bf16, kind="ExternalInput")
    wkt_d = nc.dram_tensor("wkt", [128, 8 * DG], bf16, kind="ExternalInput")
    wvt_d = nc.dram_tensor("wvt", [128, 8 * DG], bf16, kind="ExternalInput")
    wot_d = nc.dram_tensor("wot", [128, 4 * D], bf16, kind="ExternalInput")
    bqc_d = nc.dram_tensor("bqc", [128, 4], f32, kind="ExternalInput")
    bkc_d = nc.dram_tensor("bkc", [128, 4], f32, kind="ExternalInput")
    out_d = nc.dram_tensor("out", [S, D], bf16, kind="ExternalOutput")

    with tile.TileContext(nc) as tc, ExitStack() as ctx:
        consts = ctx.enter_context(tc.tile_pool(name="consts", bufs=1))
        cache = ctx.enter_context(tc.tile_pool(name="cache", bufs=1))
        xt_pool = ctx.enter_context(tc.tile_pool(name="xtp", bufs=2))
        qt_pool = ctx.enter_context(tc.tile_pool(name="qtp", bufs=3))
        work = ctx.enter_context(tc.tile_pool(name="work", bufs=1))
        ppool = ctx.enter_context(tc.tile_pool(name="pp", bufs=2, space="PSUM"))
        pscore = ctx.enter_context(tc.tile_pool(name="ps", bufs=2, space="PSUM"))
        pout2 = ctx.enter_context(tc.tile_pool(name="po", bufs=2, space="PSUM"))

        # ---- HAM warmup: matmuls on a junk tile keep the PE busy from
        # preamble-end (~7.3us) until the first projection matmuls flow,
        # so HAM un-throttles (K=8/8) before real work.
        junk_t = consts.tile([128, 512], bf16)
        nc.vector.memset(junk_t, 0.0)
        for w in range(48):
            # warmups from the score-PSUM ring bridge the whole input-DMA
            # window (~7.3-20us) so the first real matmuls run at K=8/8
            # instead of re-throttled 1.2GHz
            pwarm = pscore.tile([128, 2, SB], f32, tag="ps", name=f"pwarm_{w}")
            nc.tensor.matmul(
                pwarm[:, 0, :], junk_t[:, 0:128], junk_t[:, 0:512],
                start=True, stop=True,
            )

        # ---- weights / constants in SBUF ----
        wq_t = consts.tile([128, 8 * DG], bf16)
        wk_t = consts.tile([128, 8 * DG], bf16)
        wv_t = consts.tile([128, 8 * DG], bf16)
        wo_t = consts.tile([128, 4 * D], bf16)
        bqc_t = consts.tile([128, 4], f32)
        bkc_t = consts.tile([128, 4], f32)

        xt_tiles = {}

        def emit_xt_load(sb):
            # single DMA (128 x 8KB lines) on sync: keeps gpsimd free for the
            # causal-mask selects; the xt-pool slot WAR naturally delays this
            # transfer until the previous-but-one block's readers finish.
            xt_sb = xt_pool.tile([128, 8 * SB], bf16, tag="xt", name=f"xt_{sb}")
            nc.sync.dma_start(xt_sb[:, :], xt_d[:, sb, :])
            xt_tiles[sb] = xt_sb

        # startup-critical loads only (xt0 + QKV weights), 2-ec chunks
        # round-robined so transfers pipeline into the first matmul groups;
        # xt1/xt2/wo are issued inside the s-block loop.
        nc.sync.dma_start(bqc_t[:, :], bqc_d[:, :])
        nc.sync.dma_start(bkc_t[:, :], bkc_d[:, :])
        xt_0 = xt_pool.tile([128, 8 * SB], bf16, tag="xt", name="xt_0")
        xt_tiles[0] = xt_0
        # xt0 halves lead both queues (every projection streams xt), then
        # dc-major wq/wk chunks: chunk c unlocks head-pair c's projections
        nc.sync.dma_start(xt_0[:, 0 : 4 * SB], xt_d[:, 0, 0 : 4 * SB])
        nc.scalar.dma_start(xt_0[:, 4 * SB : 8 * SB], xt_d[:, 0, 4 * SB : 8 * SB])
        for c in range(4):
            sl = slice(c * 1024, (c + 1) * 1024)
            nc.sync.dma_start(wq_t[:, sl], wqt_d[:, sl])
            nc.scalar.dma_start(wk_t[:, sl], wkt_d[:, sl])
            nc.gpsimd.dma_start(wv_t[:, c * 2 * DG : (c + 1) * 2 * DG],
                                wvt_d[:, c * 2 * DG : (c + 1) * 2 * DG])

        # ---- persistent K/V caches ----
        kt_all = cache.tile([128, 4, S], bf16)       # [d within pair chunk, pair, t]
        # [t within chunk, tchunk, head, ones(64)|v(64)]
        v_aug = cache.tile([128, 16, NHL, 128], bf16)
        for tcm in range(16):
            nc.vector.memset(v_aug[:, tcm, :, 0:DH], 1.0)

        qt_tiles = {}
        ao_tiles = {}

        # ---- work-item emitters (each is one PSUM-group of tensor work) ----
        def emit_proj_q(sb, dc):
            xt_sb = xt_tiles[sb]
            if dc == 0:
                qt_tiles[sb] = qt_pool.tile(
                    [128, 4, SB], bf16, tag="qt", name=f"qt_{sb}"
                )
            qt_sb = qt_tiles[sb]
            pq = ppool.tile([128, SB], f32, tag="pp", name=f"pq_{sb}_{dc}")
            for ec in range(8):
                nc.tensor.matmul(
                    pq, wq_t[:, dc * 1024 + ec * 128 : dc * 1024 + ec * 128 + 128],
                    xt_sb[:, ec * SB : (ec + 1) * SB],
                    start=(ec == 0), stop=(ec == 7),
                )
            # bias-add on scalar: keeps the vector queue free so fill-group
            # PSUM slots release promptly (ring-release stalls otherwise)
            nc.scalar.activation(
                qt_sb[:, dc, :], pq, AF.Identity, bias=bqc_t[:, dc : dc + 1]
            )

        def emit_proj_k(sb, dc):
            xt_sb = xt_tiles[sb]
            s0 = sb * SB
            pk = ppool.tile([128, SB], f32, tag="pp", name=f"pk_{sb}_{dc}")
            for ec in range(8):
                nc.tensor.matmul(
                    pk, wk_t[:, dc * 1024 + ec * 128 : dc * 1024 + ec * 128 + 128],
                    xt_sb[:, ec * SB : (ec + 1) * SB],
                    start=(ec == 0), stop=(ec == 7),
                )
            nc.scalar.activation(
                kt_all[:, dc, s0 : s0 + SB], pk, AF.Identity,
                bias=bkc_t[:, dc : dc + 1],
            )

        def emit_proj_v(sb, tsub):
            xt_sb = xt_tiles[sb]
            tcg = 4 * sb + tsub
            pv = ppool.tile([128, NHL, DH], f32, tag="pp", name=f"pv_{sb}_{tsub}")
            for ec in range(8):
                nc.tensor.matmul(
                    pv, xt_sb[:, ec * SB + tsub * 128 : ec * SB + tsub * 128 + 128],
                    wv_t[:, ec * DG : (ec + 1) * DG],
                    start=(ec == 0), stop=(ec == 7),
                )
            nc.vector.tensor_copy(v_aug[:, tcg, :, DH:128], pv[:, :, :])

        def emit_outproj(sb, sc, oh, idx=0):
            s0 = sb * SB
            ao = ao_tiles[sb]
            last = sb == NSB - 1
            # the final two groups go in 256-col halves so the copy+store of
            # half A overlaps the matmuls of half B (shorter serial tail)
            halves = 2 if (last and idx >= 6) else 1
            hw = 512 // halves
            for h2 in range(halves):
                o0 = oh * 512 + h2 * hw
                pool = pout2 if (last and (idx + h2) % 2 == 1) else ppool
                po = pool.tile(
                    [128, hw], f32, tag=("po" if pool is pout2 else "pp"),
                    name=f"pop_{sb}_{sc}_{oh}_{h2}",
                )
                for p in range(4):
                    nc.tensor.matmul(
                        po,
                        ao[p][:, ts(sc, 128)],
                        wo_t[:, p * D + o0 : p * D + o0 + hw],
                        start=(p == 0), stop=(p == 3),
                    )
                po_sb = work.tile([128, hw], bf16, tag="posb", bufs=3)
                if last:
                    # tail: alternate scalar/vector copies so they pipeline
                    if (idx + h2) % 2 == 0:
                        nc.scalar.copy(po_sb, po)
                    else:
                        nc.vector.tensor_copy(po_sb, po)
                else:
                    nc.vector.tensor_copy(po_sb, po)
                (nc.gpsimd if (last and idx % 2 == 1) else nc.sync).dma_start(
                    out_d[s0 + 128 * sc : s0 + 128 * (sc + 1), o0 : o0 + hw],
                    po_sb,
                )

        def proj_items(sb):
            items = []
            for dc in range(4):
                items.append(lambda sb=sb, dc=dc: emit_proj_q(sb, dc))
            for dc in range(4):
                items.append(lambda sb=sb, dc=dc: emit_proj_k(sb, dc))
            for tsub in range(4):
                items.append(lambda sb=sb, tsub=tsub: emit_proj_v(sb, tsub))
            return items

        def outproj_items(sb):
            return [
                lambda sb=sb, sc=sc, oh=oh, i=i: emit_outproj(sb, sc, oh, i)
                for i, (sc, oh) in enumerate(
                    (sc, oh) for sc in range(4) for oh in range(2)
                )
            ]

        # full proj(0) ahead of attention(0): during the input-DMA window
        # the PE has ~20us of projection work that pipelines with the 2-ec
        # chunk arrivals, so it never idles waiting for attention deps.
        p0 = proj_items(0)
        for it in [p0[0], p0[4], p0[8], p0[9], p0[10], p0[11],
                   p0[1], p0[5], p0[2], p0[6], p0[3], p0[7]]:
            it()

        # fill-schedule per attention phase: projections go to the early
        # (exp-light, PE-idle) phases, all out-projections to the exp-bound
        # last phase.
        p3 = proj_items(3)
        fills = {
            0: proj_items(1),
            1: proj_items(2) + p3[:2],
            2: p3[2:],
            3: [],
        }

        for sb in range(NSB):
            s0 = sb * SB
            nkc = 4 * sb + 4
            qt_sb = qt_tiles[sb]

            if sb == 0:
                emit_xt_load(1)
                emit_xt_load(2)
                nc.sync.dma_start(wo_t[:, :], wot_d[:, :])
            elif sb == 1:
                emit_xt_load(3)
            fill = fills[sb]
            if sb == 3:
                fill = (
                    fill + outproj_items(0) + outproj_items(1)
                    + outproj_items(2)[:4]
                )
            total_chunks = 4 * nkc
            # even spread: emit fill item i after chunk floor((i+1)*T/(n+1));
            # sb=0 fills need xt1, which lands after the critical loads - bias
            # them into the second half so they never block attention(0)
            if sb == 0 and fill:
                emit_at = [
                    8 + (i + 1) * (total_chunks - 8) // (len(fill) + 1)
                    for i in range(len(fill))
                ]
            else:
                emit_at = [
                    (i + 1) * total_chunks // (len(fill) + 1)
                    for i in range(len(fill))
                ]
            fill_i = 0
            chunk_i = 0

            ao_tiles[sb] = []
            for p in range(4):
                out2 = [
                    pout2.tile([128, SB], f32, tag="po", name=f"out2_{hh}")
                    for hh in range(2)
                ]
                prev = None  # (exp tile, col offset, key chunk)
                for kc in range(nkc):
                    j = kc - 4 * sb  # >= 0 on diagonal chunks
                    c0 = 128 * j if j > 0 else 0
                    ps_t = pscore.tile([128, 2, SB], f32, tag="ps")
                    for hh in range(2):
                        r0 = 64 * hh
                        nc.tensor.matmul(
                            ps_t[:, hh, c0:SB],
                            kt_all[r0 : r0 + 64, p, ts(kc, 128)],
                            qt_sb[r0 : r0 + 64, p, c0:SB],
                            start=True, stop=True,
                        )
                    if prev is not None:
                        pex, pc0, pkc = prev
                        for hh in range(2):
                            nc.tensor.matmul(
                                out2[hh][:, pc0:SB],
                                v_aug[:, pkc, 2 * p + hh, :],
                                pex[:, hh, pc0:SB],
                                start=(pkc == 0), stop=False,
                            )
                    ex = work.tile([128, 2, SB], bf16, tag="expt", bufs=4)
                    nc.scalar.activation(
                        ex[:, :, c0:SB], ps_t[:, :, c0:SB], AF.Exp, scale=SCALE
                    )
                    if j >= 0:
                        # causal mask: zero ex[k, hh, q] where q < k within the
                        # 128x128 diagonal block (iota = col - chan, keep >= 0)
                        nc.gpsimd.affine_select(
                            out=ex[:, :, c0 : c0 + 128],
                            in_=ex[:, :, c0 : c0 + 128],
                            compare_op=mybir.AluOpType.is_ge,
                            fill=0.0,
                            base=0,
                            pattern=[[0, 2], [1, 128]],
                            channel_multiplier=-1,
                        )
                    prev = (ex, c0, kc)
                    chunk_i += 1
                    while fill_i < len(fill) and chunk_i >= emit_at[fill_i]:
                        fill[fill_i]()
                        fill_i += 1
                # final attn@V for the last key chunk
                pex, pc0, pkc = prev
                for hh in range(2):
                    nc.tensor.matmul(
                        out2[hh][:, pc0:SB],
                        v_aug[:, pkc, 2 * p + hh, :],
                        pex[:, hh, pc0:SB],
                        start=(pkc == 0), stop=True,
                    )

                # ---- normalization: Z sits broadcast on partitions 0-63 of
                # the same PSUM tile (ones block leads the AV stationary).
                ao_p = work.tile([128, SB], bf16, tag=f"ao{p}", bufs=4)
                for hh in range(2):
                    rbc = work.tile([64, SB], f32, tag="rbc", bufs=2)
                    nc.vector.reciprocal_approx_fast(rbc, out2[hh][0:64, :])
                    nc.vector.tensor_mul(
                        ao_p[64 * hh : 64 * hh + 64, :], out2[hh][64:128, :], rbc
                    )
                ao_tiles[sb].append(ao_p)
                for _ in range(2):
                    if fill_i < len(fill):
                        fill[fill_i]()
                        fill_i += 1

            while fill_i < len(fill):
                fill[fill_i]()
                fill_i += 1

        # tail: four held-back outproj(2) groups (no dependency on the last
        # norm) keep the PE busy while the DVE normalizes p=3, then the final
        # out-projections follow.
        for it in outproj_items(2)[4:] + outproj_items(NSB - 1):
            it()

    nc.compile()
    return nc


def _pack8(a, dtype):
    # [1024, N] row-major -> [128, 8*N]: row p holds chunks
    # {p, 128+p, ..., 896+p} concatenated (contiguous DMA lines per row)
    n = a.shape[1]
    return np.ascontiguousarray(
        a.reshape(8, 128, n).transpose(1, 0, 2).reshape(128, 8 * n)
    ).astype(dtype)


def _pack8_dc(a, dtype):
    # [1024, 512] -> [128, 4096] dc-major: [p, dc*1024 + ec*128 + r]
    return np.ascontiguousarray(
        a.reshape(8, 128, 4, 128).transpose(1, 2, 0, 3).reshape(128, 4096)
    ).astype(dtype)


def _pack_xt(xt, dtype):
    # x[b].T [1024, 2048] -> [128, 4, 4096]: [p, sb, ec*512+i]
    return np.ascontiguousarray(
        xt.reshape(8, 128, 4, 512).transpose(1, 2, 0, 3).reshape(128, 4, 4096)
    ).astype(dtype)


def _prepare_core_inputs(x, Wq, bq, Wk, bk, Wv):
    """Build per-core input maps. Core c: b = c // 2, g = c % 2."""
    import ml_dtypes

    BF = ml_dtypes.bfloat16
    maps = []
    xt = [_pack_xt(np.ascontiguousarray(x[b].T), BF) for b in range(B)]
    wq_s, wk_s, wv_s, bq_s, bk_s = [], [], [], [], []
    for g in range(HG):
        sl = slice(g * DG, (g + 1) * DG)
        wq_s.append(_pack8_dc(np.ascontiguousarray(Wq[sl, :].T), BF))
        wk_s.append(_pack8_dc(np.ascontiguousarray(Wk[sl, :].T), BF))
        wv_s.append(_pack8(np.ascontiguousarray(Wv[sl, :].T), BF))
        # per-dim bias columns: [128, 4] = bias[dc*128 + r] at [r, dc]
        bq_s.append(np.ascontiguousarray(bq[sl].reshape(4, 128).T).astype(np.float32))
        bk_s.append(np.ascontiguousarray(bk[sl].reshape(4, 128).T).astype(np.float32))
    for c in range(B * HG):
        b, g = c // HG, c % HG
        maps.append({
            "xt": xt[b],
            "wqt": wq_s[g], "wkt": wk_s[g], "wvt": wv_s[g],
            "wot": None,  # filled by caller (needs Wo)
            "bqc": bq_s[g], "bkc": bk_s[g],
        })
    return maps


def kernel(x, Wq, bq, Wk, bk, Wv, bv, Wo, bo):
    from concourse.bass_utils import run_bass_kernel_spmd

    x = np.asarray(x, dtype=np.float32)
    Wq, bq = np.asarray(Wq, np.float32), np.asarray(bq, np.float32)
    Wk, bk = np.asarray(Wk, np.float32), np.asarray(bk, np.float32)
    Wv, bv = np.asarray(Wv, np.float32), np.asarray(bv, np.float32)
    Wo, bo = np.asarray(Wo, np.float32), np.asarray(bo, np.float32)

    if "nc" not in _CACHE:
        _CACHE["nc"] = _build_nc()
    nc = _CACHE["nc"]

    maps = _prepare_core_inputs(x, Wq, bq, Wk, bk, Wv)
    wot = [_pack_wo(Wo, g) for g in range(HG)]
    for c in range(B * HG):
        maps[c]["wot"] = wot[c % HG]

    res = run_bass_kernel_spmd(nc, maps, list(range(B * HG)))
    _CACHE["last_results"] = res

    # V-bias folds out of attention exactly (softmax rows sum to 1)
    bo_eff = bo + bv @ Wo.T

    out = np.empty((B, S, D), dtype=np.float32)
    for b in range(B):
        out[b] = (
            res.results[2 * b]["out"].astype(np.float32)
            + res.results[2 * b + 1]["out"].astype(np.float32)
            + bo_eff
        )
    return out


def _pack_wo(Wo, g):
    import ml_dtypes

    w = np.ascontiguousarray(Wo.T[g * DG : (g + 1) * DG, :])  # [512, 1024]
    return np.ascontiguousarray(
        w.reshape(4, 128, D).transpose(1, 0, 2).reshape(128, 4 * D)
    ).astype(ml_dtypes.bfloat16)
